# revision 41
# baseline (speedup 1.0000x reference)
"""AttentionSimilarity kernel for 8 TRN2 NeuronCores.

Reference computation (B=2, S=2048, D=768, H=12, Hd=64):
    q = (x @ Wq.T + bq)  -> [B,S,H,Hd]
    k = (x @ Wk.T + bk)  -> [B,S,H,Hd]
    scores = einsum("bqhd,bkhd->bhqk", q, k) / sqrt(Hd)
    out = softmax(scores, -1).mean(axis=1)   -> [B,S,S]

Sharding: data-parallel over B (2 groups of 4 cores); within a group each
core owns a 512-row q-slice and computes all 12 heads for that slice.
k-projection is replicated inside the group.  fp16 on-device (fp32 PSUM
accumulation), fp16 out upcast to fp32 on host.

Per-core schedule.  ScalarE (softmax exp, 1 elem/cycle/lane, no perf
modes) is the critical engine; everything else is arranged around
keeping it saturated and pulling work from DVE to the idle engines:
  - Projections in transposed layout (qT [768,512], kT [768,2048]),
    PSUM tiles [128,512], software-pipelined one chunk ahead of the
    head-pairs that consume them.  Bias-add + PSUM->SBUF copy on DVE
    (1x tensor_scalar, PSUM source); the 3 prologue units ride ScalarE
    while it is otherwise idle during the DMA ramp.
  - Scores per (head-pair, q-tile, k-half): two [128,1024] fp32 PSUM
    tiles filled by interleaved K=64 matmuls (PSUM: scp 3x2 banks +
    ppp 2x1 = 8).
  - exp on ScalarE per (head, k-half) with fused per-row sum via
    accum_out (~1225ns per half-plane); fp16 E plane in SBUF.  8 planes
    (APPROX, scattered one-per-(hp,qt) region so the scalar bubbles
    spread out) instead use a Schraudolph fast-exp on DVE - dual-op
    tensor_scalar into an int16-bitcast view of the E tile - with the
    row-sum from a 4x in-place copy-with-accum_out; ~1.8% per-plane
    softmax error, ~0.3% end-to-end (gate is 2e-2).
  - DVE: d12 = (d0+d1)*12 in one dual-op tensor_scalar, reciprocal,
    then Es = E*r12 (4x tensor_scalar).  acc += Es runs on DVE (2x
    tensor_tensor) for 16 planes and on the otherwise-idle gpsimd
    engine for 28 planes (TensorTensor is the only elementwise opcode
    the Pool-engine ISA accepts; fused scalar_tensor_tensor has no DVE
    perf-mode uops; at the HW-measured ~4.4us/plane Q7 cost the 28
    planes keep Pool just under ScalarE, so the offload cannot bind).
  - Ramp-critical input DMA issues round-robin over the SP, gpsimd and
    (pre-exp idle) ScalarE queues; gpsimd's SWDGE issues all complete
    before its first accumulate, so they never contend.  Final planes
    write f16 output in halves (DMA of half 0 overlaps compute of
    half 1) on the SP queue; host upcasts to f32.
"""

import numpy as np

B = 2
S = 2048
D = 768
H = 12
HD = 64
P = 128
DI = D // P            # 6 chunks of the contraction dim
NCORES = 8
QPC = S // 4           # 512 q rows per core
QT = QPC // P          # 4 q-tiles per core
KC = S // 512          # 4 k chunks of 512
NHALF = 2              # two k-halves of 1024

# Planes (hp, qt, i) whose accumulate step runs as a fused
# scalar_tensor_tensor on the gpsimd (Pool) engine instead of DVE.
POOL_ACC = ({(hp, qt, 0) for hp in range(1, 5) for qt in range(4)}
            | {(2, qt, 1) for qt in range(4)}
            | {(0, 0, 1), (0, 2, 1), (3, 1, 1), (3, 3, 1)})

# Planes (hp, qt, i) whose softmax exp runs as a Schraudolph fast-exp on
# DVE (bit-trick: f16 bits = trunc(z*1024 + (15-C)*1024), z = log2 e(x));
# relieves the saturated ScalarE.  Per-plane softmax rel err ~1.8%, but
# only ~n/12-th of each output element is affected -> ~0.3% overall.
APPROX = {(1, 0, 0), (1, 2, 0), (2, 1, 0), (2, 3, 0),
          (3, 0, 0), (3, 2, 0), (4, 1, 0), (4, 3, 0)}

# Projection units whose bias-add + PSUM->SBUF copy runs on ScalarE
# (in its slack windows) instead of DVE.  ("q", t) = q-projection chunk t;
# (t, rc) = k-projection chunk t, k-range rc.
SCALAR_PROJ = {(0, 0), (0, 1), ("q", 0), (2, 1)}

# Exact-exp planes whose softmax denominator comes from a DVE 4x
# copy-with-accum instead of the scalar activation's accum_out (saves
# 2x187ns of ScalarE per plane for 594ns of DVE).
DVE_DSUM: set = set()

_LOG2E = 1.4426950408889634
_K1 = float(0.125 * _LOG2E * 1024.0)
_K2 = float((15.0 - 0.0434) * 1024.0)

_BUILT = None


def _build():
    global _BUILT
    if _BUILT is not None:
        return _BUILT

    import concourse.bass as bass  # noqa: F401
    import concourse.mybir as mybir
    import concourse.tile as tile
    from concourse import bacc

    f32 = mybir.dt.float32
    f16 = mybir.dt.float16
    i16 = mybir.dt.int16
    Alu = mybir.AluOpType
    Act = mybir.ActivationFunctionType

    nc = bacc.Bacc("TRN2", target_bir_lowering=False, debug=False,
                   num_devices=NCORES)

    xT = nc.dram_tensor("xT", [D, S], f16, kind="ExternalInput").ap()
    xqT = nc.dram_tensor("xqT", [D, QPC], f16, kind="ExternalInput").ap()
    wqT = nc.dram_tensor("wqT", [D, D], f16, kind="ExternalInput").ap()
    wkT = nc.dram_tensor("wkT", [D, D], f16, kind="ExternalInput").ap()
    bq = nc.dram_tensor("bq", [D], f32, kind="ExternalInput").ap()
    bk = nc.dram_tensor("bk", [D], f32, kind="ExternalInput").ap()
    out = nc.dram_tensor("out", [QPC, S], f16, kind="ExternalOutput").ap()

    xT_r = xT.rearrange("(c p) s -> p c s", p=P)
    xqT_r = xqT.rearrange("(c p) s -> p c s", p=P)
    wqT_r = wqT.rearrange("(c p) d -> p c d", p=P)
    wkT_r = wkT.rearrange("(c p) d -> p c d", p=P)
    bq_r = bq.rearrange("(c p) -> p c", p=P)
    bk_r = bk.rearrange("(c p) -> p c", p=P)

    with tile.TileContext(nc) as tc:
        import contextlib
        with contextlib.ExitStack() as ctx:
            consts = ctx.enter_context(tc.tile_pool(name="consts", bufs=1))
            scp = ctx.enter_context(
                tc.tile_pool(name="scp", bufs=3, space="PSUM"))
            ppp = ctx.enter_context(
                tc.tile_pool(name="ppp", bufs=2, space="PSUM"))
            epool = ctx.enter_context(tc.tile_pool(name="epool", bufs=10))
            espool = ctx.enter_context(tc.tile_pool(name="espool", bufs=8))
            dpool = ctx.enter_context(tc.tile_pool(name="dpool", bufs=8))
            accp = ctx.enter_context(tc.tile_pool(name="accp", bufs=1))
            outp = ctx.enter_context(tc.tile_pool(name="outp", bufs=3))

            # ---- persistent SBUF tensors ----
            xT_sb = consts.tile([P, DI, S], f16, tag="xT")
            xq_sb = consts.tile([P, DI, QPC], f16, tag="xq")
            wq_sb = consts.tile([P, DI, D], f16, tag="wq")
            wk_sb = consts.tile([P, DI, D], f16, tag="wk")
            bq_sb = consts.tile([P, DI], f32, tag="bq")
            bk_sb = consts.tile([P, DI], f32, tag="bk")
            kT_sb = consts.tile([P, DI, S], f16, tag="kT")
            qT_sb = consts.tile([P, DI, QPC], f16, tag="qT")
            warm = consts.tile([P, 1], f32, tag="warm")
            accs = [accp.tile([P, S], f16, tag=f"acc{qt}", name=f"acc{qt}")
                    for qt in range(QT)]

            # Load the exp table set while the DMA ramp runs so the first
            # real exp doesn't stall ~2.7us on ACT_TABLE_LOAD.
            nc.scalar.activation(out=warm, in_=warm, func=Act.Exp, scale=1.0)

            # ---- DMAs, ordered by first use; ~500ns per dma_start issue,
            # so round-robin the ramp-critical loads across the SP, Pool
            # and (still idle) Activation queues.
            # Critical path: kT[0] rc0/rc1 + qT[0] -> first score planes.
            qs = [nc.sync, nc.gpsimd, nc.scalar]
            crit = []
            for di in range(DI):
                crit.append((xT_sb[:, di, 0:512], xT_r[:, di, 0:512]))
                crit.append((wk_sb[:, di, 0:P], wkT_r[:, di, 0:P]))
            crit.append((bk_sb, bk_r))
            for di in range(DI):
                crit.append((xq_sb[:, di, :], xqT_r[:, di, :]))
                crit.append((wq_sb[:, di, 0:P], wqT_r[:, di, 0:P]))
            crit.append((bq_sb, bq_r))
            for n, (dst, src) in enumerate(crit):
                qs[n % 3].dma_start(out=dst, in_=src)
            rest = []
            for di in range(DI):
                rest.append((xT_sb[:, di, 512:1024], xT_r[:, di, 512:1024]))
            for di in range(DI):
                rest.append((xT_sb[:, di, 1024:1536], xT_r[:, di, 1024:1536]))
                rest.append((xT_sb[:, di, 1536:2048], xT_r[:, di, 1536:2048]))
            for di in range(DI):
                rest.append((wk_sb[:, di, P:D], wkT_r[:, di, P:D]))
                rest.append((wq_sb[:, di, P:D], wqT_r[:, di, P:D]))
            for n, (dst, src) in enumerate(rest):
                qs[n % 2].dma_start(out=dst, in_=src)

            def proj_q(t):
                """Project qT chunk t (transposed layout)."""
                on_scalar = ("q", t) in SCALAR_PROJ
                tsl = slice(t * P, (t + 1) * P)
                ps = ppp.tile([P, 512], f32, tag="pp")
                for di in range(DI):
                    nc.tensor.matmul(ps, wq_sb[:, di, tsl], xq_sb[:, di, :],
                                     start=(di == 0), stop=(di == DI - 1))
                if on_scalar:
                    nc.scalar.activation(out=qT_sb[:, t, :], in_=ps,
                                         func=Act.Identity,
                                         bias=bq_sb[:, t:t + 1], scale=1.0)
                else:
                    nc.vector.tensor_scalar_add(
                        out=qT_sb[:, t, :], in0=ps, scalar1=bq_sb[:, t:t + 1])

            def proj_k(t, rc):
                """Project kT chunk t, k-range rc (transposed layout)."""
                on_scalar = (t, rc) in SCALAR_PROJ
                tsl = slice(t * P, (t + 1) * P)
                rs = slice(rc * 512, (rc + 1) * 512)
                ps2 = ppp.tile([P, 512], f32, tag="pp")
                for di in range(DI):
                    nc.tensor.matmul(ps2, wk_sb[:, di, tsl],
                                     xT_sb[:, di, rs],
                                     start=(di == 0), stop=(di == DI - 1))
                if on_scalar:
                    nc.scalar.activation(out=kT_sb[:, t, rs], in_=ps2,
                                         func=Act.Identity,
                                         bias=bk_sb[:, t:t + 1], scale=1.0)
                else:
                    nc.vector.tensor_scalar_add(
                        out=kT_sb[:, t, rs], in0=ps2,
                        scalar1=bk_sb[:, t:t + 1])

            def proj_parts(t, qt):
                """Spread projection of chunk t across the 4 qt steps."""
                if t >= DI:
                    return
                if qt == 0:
                    proj_q(t)
                    proj_k(t, 0)
                elif qt == 1:
                    proj_k(t, 1)
                elif qt == 2:
                    proj_k(t, 2)
                else:
                    proj_k(t, 3)

            # prologue: only what the very first score planes need —
            # kT[0] rc0/rc1 + qT[0]; ScalarE is idle during the DMA ramp,
            # so these three bias-copies ride it.
            proj_k(0, 0)
            proj_k(0, 1)
            proj_q(0)

            for hp in range(DI):
                t = hp
                tsl = slice(t * P, (t + 1) * P)
                for qt in range(QT):
                    qsl = slice(qt * P, (qt + 1) * P)
                    acc = accs[qt]
                    # E planes + denominators for the two heads of the pair
                    Epl = [epool.tile([P, S], f16, tag="E", name=f"E_{hp}_{qt}_{i}")
                           for i in range(2)]
                    dt_ = [dpool.tile([P, NHALF], f32, tag="d",
                                       name=f"d_{hp}_{qt}_{i}")
                           for i in range(2)]
                    for j in range(NHALF):
                        pss = [scp.tile([P, 1024], f32, tag="sc",
                                          name=f"sc_{hp}_{qt}_{j}_{i}")
                               for i in range(2)]
                        for rc2 in range(2):
                            rs = slice(j * 1024 + rc2 * 512,
                                       j * 1024 + (rc2 + 1) * 512)
                            ps_sl = slice(rc2 * 512, (rc2 + 1) * 512)
                            for i in range(2):  # head pair, interleaved
                                po = i * HD
                                nc.tensor.matmul(
                                    pss[i][:, ps_sl],
                                    qT_sb[po:po + HD, t, qsl],
                                    kT_sb[po:po + HD, t, rs],
                                    start=True, stop=True)
                        for i in range(2):
                            jsl = slice(j * 1024, (j + 1) * 1024)
                            if (hp, qt, i) in APPROX:
                                # fast-exp on DVE: f16 bits are an affine
                                # function of log2(E); trunc-to-int16 cast
                                # + bitcast produce exp() to ~2% rel.
                                nc.vector.tensor_scalar(
                                    out=Epl[i][:, jsl].bitcast(i16),
                                    in0=pss[i], scalar1=_K1, scalar2=_K2,
                                    op0=Alu.mult, op1=Alu.add)
                            elif (hp, qt, i) in DVE_DSUM:
                                nc.scalar.activation(
                                    out=Epl[i][:, jsl],
                                    in_=pss[i],
                                    func=Act.Exp, scale=0.125)
                            else:
                                nc.scalar.activation(
                                    out=Epl[i][:, jsl],
                                    in_=pss[i],
                                    func=Act.Exp, scale=0.125,
                                    accum_out=dt_[i][:, j:j + 1])
                        if hp == 0 and qt == 0 and j == 0:
                            proj_k(0, 2)
                            proj_k(0, 3)
                    for i in range(2):
                        h = 2 * hp + i
                        # d12 = (d0 + d1) * 12, then r12 = 1/d12: folds the
                        # head-mean into the per-row softmax scale.
                        d12 = dpool.tile([P, 1], f32, tag="dd")
                        if (hp, qt, i) in APPROX or (hp, qt, i) in DVE_DSUM:
                            # row-sum of the E plane via an in-place 4x
                            # copy with accum_out.
                            dfull = dpool.tile([P, 1], f32, tag="df")
                            nc.vector.tensor_scalar(
                                out=Epl[i], in0=Epl[i], scalar1=1.0,
                                scalar2=None, op0=Alu.mult, op1=Alu.add,
                                accum_out=dfull)
                            nc.vector.tensor_scalar_mul(
                                out=d12, in0=dfull, scalar1=float(H))
                        else:
                            nc.vector.tensor_scalar(
                                out=d12, in0=dt_[i][:, 0:1],
                                scalar1=dt_[i][:, 1:2], scalar2=float(H),
                                op0=Alu.add, op1=Alu.mult)
                        r12 = dpool.tile([P, 1], f32, tag="r12")
                        nc.vector.reciprocal(out=r12, in_=d12)
                        on_pool = (hp, qt, i) in POOL_ACC
                        if h == 0:
                            nc.vector.tensor_scalar_mul(
                                out=acc, in0=Epl[i], scalar1=r12)
                        elif h == H - 1:
                            # out = E*r12 + acc, in halves so the first
                            # half's DMA overlaps the second half's compute
                            # (f16; host upcasts)
                            ot = outp.tile([P, S], f16, tag="ot")
                            for jo in range(2):
                                js = slice(jo * 1024, (jo + 1) * 1024)
                                Es = espool.tile([P, 1024], f16, tag="Esh")
                                nc.vector.tensor_scalar_mul(
                                    out=Es, in0=Epl[i][:, js], scalar1=r12)
                                nc.vector.tensor_tensor(
                                    out=ot[:, js], in0=acc[:, js], in1=Es,
                                    op=Alu.add)
                                nc.sync.dma_start(out=out[qsl, js],
                                                  in_=ot[:, js])
                        else:
                            # Es = E*r12 on DVE (4x tensor_scalar); the
                            # accumulate acc += Es runs on DVE (2x TT) or,
                            # for POOL_ACC planes, on the otherwise-idle
                            # gpsimd engine (TT is the only elementwise
                            # opcode the Pool engine ISA accepts).
                            Es = espool.tile([P, S], f16, tag="Es")
                            nc.vector.tensor_scalar_mul(
                                out=Es, in0=Epl[i], scalar1=r12)
                            eng = nc.gpsimd if on_pool else nc.vector
                            eng.tensor_tensor(
                                out=acc, in0=acc, in1=Es, op=Alu.add)
                    # software-pipeline the next projection chunk:
                    # hp 0 finishes chunk 1; hp t-1 projects chunk t.
                    if hp == 0:
                        if qt == 0:
                            proj_q(1)
                            proj_k(1, 0)
                        else:
                            proj_k(1, qt)
                    else:
                        proj_parts(hp + 1, qt)

    nc.compile()
    _BUILT = (nc,)
    return _BUILT


def make_in_maps(x, Wq, bq, Wk, bk):
    f16 = np.float16
    x = np.asarray(x, dtype=np.float32)
    wqT = np.ascontiguousarray(np.asarray(Wq, np.float32).T).astype(f16)
    wkT = np.ascontiguousarray(np.asarray(Wk, np.float32).T).astype(f16)
    bq = np.asarray(bq, np.float32)
    bk = np.asarray(bk, np.float32)
    in_maps = []
    for c in range(NCORES):
        b, qc = c // 4, c % 4
        xTb = np.ascontiguousarray(x[b].T).astype(f16)      # [768, 2048]
        xqTc = np.ascontiguousarray(xTb[:, qc * QPC:(qc + 1) * QPC])
        in_maps.append({
            "xT": xTb,
            "xqT": xqTc,
            "wqT": wqT,
            "wkT": wkT,
            "bq": bq,
            "bk": bk,
        })
    return in_maps


def run(x, Wq, bq, Wk, bk, trace=False, **trace_kwargs):
    from concourse.bass_utils import run_bass_kernel_spmd
    (nc,) = _build()
    in_maps = make_in_maps(x, Wq, bq, Wk, bk)
    res = run_bass_kernel_spmd(
        nc, in_maps, core_ids=list(range(NCORES)), trace=trace,
        **trace_kwargs)
    outp = np.zeros((B, S, S), np.float32)
    for c in range(NCORES):
        b, qc = c // 4, c % 4
        outp[b, qc * QPC:(qc + 1) * QPC, :] = \
            res.results[c]["out"].astype(np.float32)
    return outp, res


def kernel(x, Wq, bq, Wk, bk):
    outp, _ = run(x, Wq, bq, Wk, bk, trace=False)
    return outp


# revision 58
# speedup vs baseline: 1.0071x; 1.0071x over previous
"""AttentionSimilarity kernel for 8 TRN2 NeuronCores.

Reference computation (B=2, S=2048, D=768, H=12, Hd=64):
    q = (x @ Wq.T + bq)  -> [B,S,H,Hd]
    k = (x @ Wk.T + bk)  -> [B,S,H,Hd]
    scores = einsum("bqhd,bkhd->bhqk", q, k) / sqrt(Hd)
    out = softmax(scores, -1).mean(axis=1)   -> [B,S,S]

Sharding: data-parallel over B (2 groups of 4 cores); within a group each
core owns a 512-row q-slice and computes all 12 heads for that slice.
k-projection is replicated inside the group.  fp16 on-device (fp32 PSUM
accumulation), fp16 out upcast to fp32 on host.

Per-core schedule.  ScalarE (softmax exp, 1 elem/cycle/lane, no perf
modes) is the critical engine; everything else is arranged around
keeping it saturated and pulling work from DVE to the idle engines:
  - Projections in transposed layout (qT [768,512], kT [768,2048]),
    PSUM tiles [128,512], software-pipelined one chunk ahead of the
    head-pairs that consume them.  Bias-add + PSUM->SBUF copy on DVE
    (1x tensor_scalar, PSUM source); the 3 prologue units ride ScalarE
    while it is otherwise idle during the DMA ramp.
  - Scores per (head-pair, q-tile, k-half): two [128,1024] fp32 PSUM
    tiles filled by interleaved K=64 matmuls (PSUM: scp 3x2 banks +
    ppp 2x1 = 8).
  - exp on ScalarE per (head, k-half) with fused per-row sum via
    accum_out (~1225ns per half-plane); fp16 E plane in SBUF.  8 planes
    (APPROX, scattered one-per-(hp,qt) region so the scalar bubbles
    spread out) instead use a Schraudolph fast-exp on DVE - dual-op
    tensor_scalar into an int16-bitcast view of the E tile - with the
    row-sum from a 4x in-place copy-with-accum_out; ~1.8% per-plane
    softmax error, ~0.3% end-to-end (gate is 2e-2).
  - DVE: d12 = (d0+d1)*12 in one dual-op tensor_scalar, reciprocal,
    then Es = E*r12 (4x tensor_scalar).  acc += Es runs on DVE (2x
    tensor_tensor) for 16 planes and on the otherwise-idle gpsimd
    engine for 28 planes (TensorTensor is the only elementwise opcode
    the Pool-engine ISA accepts; fused scalar_tensor_tensor has no DVE
    perf-mode uops; at the HW-measured ~4.4us/plane Q7 cost the 28
    planes keep Pool just under ScalarE, so the offload cannot bind).
  - Ramp-critical input DMA issues round-robin over the SP, gpsimd and
    (pre-exp idle) ScalarE queues; gpsimd's SWDGE issues all complete
    before its first accumulate, so they never contend.  Final planes
    write f16 output in halves (DMA of half 0 overlaps compute of
    half 1) on the SP queue; host upcasts to f32.
"""

import numpy as np

B = 2
S = 2048
D = 768
H = 12
HD = 64
P = 128
DI = D // P            # 6 chunks of the contraction dim
NCORES = 8
QPC = S // 4           # 512 q rows per core
QT = QPC // P          # 4 q-tiles per core
KC = S // 512          # 4 k chunks of 512
NHALF = 2              # two k-halves of 1024

# Planes (hp, qt, i) whose accumulate step runs as a fused
# scalar_tensor_tensor on the gpsimd (Pool) engine instead of DVE.
POOL_ACC = ({(hp, qt, 0) for hp in range(1, 5) for qt in range(4)}
            | {(2, qt, 1) for qt in range(4)}
            | {(0, 0, 1), (0, 2, 1), (3, 1, 1), (3, 3, 1)})

# Planes (hp, qt, i) whose softmax exp runs as a Schraudolph fast-exp on
# DVE (bit-trick: f16 bits = trunc(z*1024 + (15-C)*1024), z = log2 e(x));
# relieves the saturated ScalarE.  Per-plane softmax rel err ~1.8%, but
# only ~n/12-th of each output element is affected -> ~0.3% overall.
APPROX = {(1, 0, 0), (1, 2, 0), (2, 1, 0), (2, 3, 0),
          (3, 0, 0), (3, 2, 0), (4, 1, 0), (4, 3, 0)}

# Projection units whose bias-add + PSUM->SBUF copy runs on ScalarE
# (in its slack windows) instead of DVE.  ("q", t) = q-projection chunk t;
# (t, rc) = k-projection chunk t, k-range rc.
SCALAR_PROJ = {(0, 0), (0, 1), ("q", 0), (2, 1)}

# Exact-exp planes whose softmax denominator comes from a DVE 4x
# copy-with-accum instead of the scalar activation's accum_out (saves
# 2x187ns of ScalarE per plane for 594ns of DVE).
DVE_DSUM: set = set()

_LOG2E = 1.4426950408889634
_K1 = float(0.125 * _LOG2E * 1024.0)
_K2 = float((15.0 - 0.0434) * 1024.0)

_BUILT = None


def _build():
    global _BUILT
    if _BUILT is not None:
        return _BUILT

    import concourse.bass as bass  # noqa: F401
    import concourse.mybir as mybir
    import concourse.tile as tile
    from concourse import bacc

    f32 = mybir.dt.float32
    f16 = mybir.dt.float16
    i16 = mybir.dt.int16
    Alu = mybir.AluOpType
    Act = mybir.ActivationFunctionType

    nc = bacc.Bacc("TRN2", target_bir_lowering=False, debug=False,
                   num_devices=NCORES)

    xT = nc.dram_tensor("xT", [D, S], f16, kind="ExternalInput").ap()
    xqT = nc.dram_tensor("xqT", [D, QPC], f16, kind="ExternalInput").ap()
    wqT = nc.dram_tensor("wqT", [D, D], f16, kind="ExternalInput").ap()
    wkT = nc.dram_tensor("wkT", [D, D], f16, kind="ExternalInput").ap()
    bq = nc.dram_tensor("bq", [D], f32, kind="ExternalInput").ap()
    bk = nc.dram_tensor("bk", [D], f32, kind="ExternalInput").ap()
    out = nc.dram_tensor("out", [QPC, S], f16, kind="ExternalOutput").ap()

    xT_r = xT.rearrange("(c p) s -> p c s", p=P)
    xqT_r = xqT.rearrange("(c p) s -> p c s", p=P)
    wqT_r = wqT.rearrange("(c p) d -> p c d", p=P)
    wkT_r = wkT.rearrange("(c p) d -> p c d", p=P)
    bq_r = bq.rearrange("(c p) -> p c", p=P)
    bk_r = bk.rearrange("(c p) -> p c", p=P)

    with tile.TileContext(nc) as tc:
        import contextlib
        with contextlib.ExitStack() as ctx:
            consts = ctx.enter_context(tc.tile_pool(name="consts", bufs=1))
            scp = ctx.enter_context(
                tc.tile_pool(name="scp", bufs=3, space="PSUM"))
            ppp = ctx.enter_context(
                tc.tile_pool(name="ppp", bufs=2, space="PSUM"))
            epool = ctx.enter_context(tc.tile_pool(name="epool", bufs=10))
            espool = ctx.enter_context(tc.tile_pool(name="espool", bufs=8))
            dpool = ctx.enter_context(tc.tile_pool(name="dpool", bufs=8))
            accp = ctx.enter_context(tc.tile_pool(name="accp", bufs=1))
            outp = ctx.enter_context(tc.tile_pool(name="outp", bufs=3))

            # ---- persistent SBUF tensors ----
            xT_sb = consts.tile([P, DI, S], f16, tag="xT")
            xq_sb = consts.tile([P, DI, QPC], f16, tag="xq")
            wq_sb = consts.tile([P, DI, D], f16, tag="wq")
            wk_sb = consts.tile([P, DI, D], f16, tag="wk")
            bq_sb = consts.tile([P, DI], f32, tag="bq")
            bk_sb = consts.tile([P, DI], f32, tag="bk")
            kT_sb = consts.tile([P, DI, S], f16, tag="kT")
            qT_sb = consts.tile([P, DI, QPC], f16, tag="qT")
            warm = consts.tile([P, 1], f32, tag="warm")
            accs = [accp.tile([P, S], f16, tag=f"acc{qt}", name=f"acc{qt}")
                    for qt in range(QT)]

            # Load the exp table set while the DMA ramp runs so the first
            # real exp doesn't stall ~2.7us on ACT_TABLE_LOAD.
            nc.scalar.activation(out=warm, in_=warm, func=Act.Exp, scale=1.0)

            # ---- DMAs, ordered by first use; ~500ns per dma_start issue,
            # so round-robin the ramp-critical loads across the SP, Pool
            # and (still idle) Activation queues.
            # Critical path: kT[0] rc0/rc1 + qT[0] -> first score planes.
            qs = [nc.sync, nc.gpsimd, nc.scalar]
            crit = []
            for di in range(DI):
                crit.append((xT_sb[:, di, 0:512], xT_r[:, di, 0:512]))
                crit.append((wk_sb[:, di, 0:P], wkT_r[:, di, 0:P]))
            crit.append((bk_sb, bk_r))
            for di in range(DI):
                crit.append((xq_sb[:, di, :], xqT_r[:, di, :]))
                crit.append((wq_sb[:, di, 0:P], wqT_r[:, di, 0:P]))
            crit.append((bq_sb, bq_r))
            for n, (dst, src) in enumerate(crit):
                qs[n % 3].dma_start(out=dst, in_=src)
            rest = []
            for di in range(DI):
                rest.append((xT_sb[:, di, 512:1024], xT_r[:, di, 512:1024]))
            for di in range(DI):
                rest.append((xT_sb[:, di, 1024:1536], xT_r[:, di, 1024:1536]))
                rest.append((xT_sb[:, di, 1536:2048], xT_r[:, di, 1536:2048]))
            for di in range(DI):
                rest.append((wk_sb[:, di, P:D], wkT_r[:, di, P:D]))
                rest.append((wq_sb[:, di, P:D], wqT_r[:, di, P:D]))
            for n, (dst, src) in enumerate(rest):
                qs[n % 2].dma_start(out=dst, in_=src)

            def proj_q(t):
                """Project qT chunk t (transposed layout)."""
                on_scalar = ("q", t) in SCALAR_PROJ
                tsl = slice(t * P, (t + 1) * P)
                ps = ppp.tile([P, 512], f32, tag="pp")
                for di in range(DI):
                    nc.tensor.matmul(ps, wq_sb[:, di, tsl], xq_sb[:, di, :],
                                     start=(di == 0), stop=(di == DI - 1))
                if on_scalar:
                    nc.scalar.activation(out=qT_sb[:, t, :], in_=ps,
                                         func=Act.Identity,
                                         bias=bq_sb[:, t:t + 1], scale=1.0)
                else:
                    nc.vector.tensor_scalar_add(
                        out=qT_sb[:, t, :], in0=ps, scalar1=bq_sb[:, t:t + 1])

            def proj_k(t, rc):
                """Project kT chunk t, k-range rc (transposed layout)."""
                on_scalar = (t, rc) in SCALAR_PROJ
                tsl = slice(t * P, (t + 1) * P)
                rs = slice(rc * 512, (rc + 1) * 512)
                ps2 = ppp.tile([P, 512], f32, tag="pp")
                for di in range(DI):
                    nc.tensor.matmul(ps2, wk_sb[:, di, tsl],
                                     xT_sb[:, di, rs],
                                     start=(di == 0), stop=(di == DI - 1))
                if on_scalar:
                    nc.scalar.activation(out=kT_sb[:, t, rs], in_=ps2,
                                         func=Act.Identity,
                                         bias=bk_sb[:, t:t + 1], scale=1.0)
                else:
                    nc.vector.tensor_scalar_add(
                        out=kT_sb[:, t, rs], in0=ps2,
                        scalar1=bk_sb[:, t:t + 1])

            def proj_parts(t, qt):
                """Spread projection of chunk t across the 4 qt steps."""
                if t >= DI:
                    return
                if qt == 0:
                    proj_q(t)
                    proj_k(t, 0)
                elif qt == 1:
                    proj_k(t, 1)
                elif qt == 2:
                    proj_k(t, 2)
                else:
                    proj_k(t, 3)

            def proj_q_part(t, c0, c1, on_scalar):
                """Project qT chunk t, q-columns [c0:c1) only."""
                tsl = slice(t * P, (t + 1) * P)
                ps = ppp.tile([P, c1 - c0], f32, tag="pp")
                for di in range(DI):
                    nc.tensor.matmul(ps, wq_sb[:, di, tsl],
                                     xq_sb[:, di, c0:c1],
                                     start=(di == 0), stop=(di == DI - 1))
                if on_scalar:
                    nc.scalar.activation(out=qT_sb[:, t, c0:c1], in_=ps,
                                         func=Act.Identity,
                                         bias=bq_sb[:, t:t + 1], scale=1.0)
                else:
                    nc.vector.tensor_scalar_add(
                        out=qT_sb[:, t, c0:c1], in0=ps,
                        scalar1=bq_sb[:, t:t + 1])

            # prologue: only what the very first score plane needs —
            # kT[0] rc0/rc1 + qT[0][:, 0:128] (the first q-tile's columns;
            # projecting all 512 up front would put 6 more K=128 matmuls
            # ahead of the first exp).  ScalarE is idle during the DMA
            # ramp, so these bias-copies ride it.
            proj_k(0, 0)
            proj_q_part(0, 0, P, on_scalar=True)
            proj_k(0, 1)
            proj_q_part(0, P, QPC, on_scalar=True)

            for hp in range(DI):
                t = hp
                tsl = slice(t * P, (t + 1) * P)
                for qt in range(QT):
                    qsl = slice(qt * P, (qt + 1) * P)
                    acc = accs[qt]
                    # E planes + denominators for the two heads of the pair
                    Epl = [epool.tile([P, S], f16, tag="E", name=f"E_{hp}_{qt}_{i}")
                           for i in range(2)]
                    dt_ = [dpool.tile([P, NHALF], f32, tag="d",
                                       name=f"d_{hp}_{qt}_{i}")
                           for i in range(2)]
                    for j in range(NHALF):
                        pss = [scp.tile([P, 1024], f32, tag="sc",
                                          name=f"sc_{hp}_{qt}_{j}_{i}")
                               for i in range(2)]
                        for rc2 in range(2):
                            rs = slice(j * 1024 + rc2 * 512,
                                       j * 1024 + (rc2 + 1) * 512)
                            ps_sl = slice(rc2 * 512, (rc2 + 1) * 512)
                            for i in range(2):  # head pair, interleaved
                                po = i * HD
                                nc.tensor.matmul(
                                    pss[i][:, ps_sl],
                                    qT_sb[po:po + HD, t, qsl],
                                    kT_sb[po:po + HD, t, rs],
                                    start=True, stop=True)
                        for i in range(2):
                            jsl = slice(j * 1024, (j + 1) * 1024)
                            if (hp, qt, i) in APPROX:
                                # fast-exp on DVE: f16 bits are an affine
                                # function of log2(E); trunc-to-int16 cast
                                # + bitcast produce exp() to ~2% rel.
                                nc.vector.tensor_scalar(
                                    out=Epl[i][:, jsl].bitcast(i16),
                                    in0=pss[i], scalar1=_K1, scalar2=_K2,
                                    op0=Alu.mult, op1=Alu.add)
                            elif (hp, qt, i) in DVE_DSUM:
                                nc.scalar.activation(
                                    out=Epl[i][:, jsl],
                                    in_=pss[i],
                                    func=Act.Exp, scale=0.125)
                            else:
                                nc.scalar.activation(
                                    out=Epl[i][:, jsl],
                                    in_=pss[i],
                                    func=Act.Exp, scale=0.125,
                                    accum_out=dt_[i][:, j:j + 1])
                        if hp == 0 and qt == 0 and j == 0:
                            proj_k(0, 2)
                            proj_k(0, 3)
                    for i in range(2):
                        h = 2 * hp + i
                        # d12 = (d0 + d1) * 12, then r12 = 1/d12: folds the
                        # head-mean into the per-row softmax scale.
                        d12 = dpool.tile([P, 1], f32, tag="dd")
                        if (hp, qt, i) in APPROX or (hp, qt, i) in DVE_DSUM:
                            # row-sum of the E plane via an in-place 4x
                            # copy with accum_out.
                            dfull = dpool.tile([P, 1], f32, tag="df")
                            nc.vector.tensor_scalar(
                                out=Epl[i], in0=Epl[i], scalar1=1.0,
                                scalar2=None, op0=Alu.mult, op1=Alu.add,
                                accum_out=dfull)
                            nc.vector.tensor_scalar_mul(
                                out=d12, in0=dfull, scalar1=float(H))
                        else:
                            nc.vector.tensor_scalar(
                                out=d12, in0=dt_[i][:, 0:1],
                                scalar1=dt_[i][:, 1:2], scalar2=float(H),
                                op0=Alu.add, op1=Alu.mult)
                        r12 = dpool.tile([P, 1], f32, tag="r12")
                        nc.vector.reciprocal(out=r12, in_=d12)
                        on_pool = (hp, qt, i) in POOL_ACC
                        if h == 0:
                            nc.vector.tensor_scalar_mul(
                                out=acc, in0=Epl[i], scalar1=r12)
                        elif h == H - 1:
                            # out = E*r12 + acc, in halves so the first
                            # half's DMA overlaps the second half's compute
                            # (f16; host upcasts)
                            ot = outp.tile([P, S], f16, tag="ot")
                            for jo in range(2):
                                js = slice(jo * 1024, (jo + 1) * 1024)
                                Es = espool.tile([P, 1024], f16, tag="Esh")
                                nc.vector.tensor_scalar_mul(
                                    out=Es, in0=Epl[i][:, js], scalar1=r12)
                                nc.vector.tensor_tensor(
                                    out=ot[:, js], in0=acc[:, js], in1=Es,
                                    op=Alu.add)
                                nc.sync.dma_start(out=out[qsl, js],
                                                  in_=ot[:, js])
                        else:
                            # Es = E*r12 on DVE (4x tensor_scalar); the
                            # accumulate acc += Es runs on DVE (2x TT) or,
                            # for POOL_ACC planes, on the otherwise-idle
                            # gpsimd engine (TT is the only elementwise
                            # opcode the Pool engine ISA accepts).
                            Es = espool.tile([P, S], f16, tag="Es")
                            nc.vector.tensor_scalar_mul(
                                out=Es, in0=Epl[i], scalar1=r12)
                            eng = nc.gpsimd if on_pool else nc.vector
                            eng.tensor_tensor(
                                out=acc, in0=acc, in1=Es, op=Alu.add)
                    # software-pipeline the next projection chunk:
                    # hp 0 finishes chunk 1; hp t-1 projects chunk t.
                    if hp == 0:
                        if qt == 0:
                            proj_q(1)
                            proj_k(1, 0)
                        else:
                            proj_k(1, qt)
                    else:
                        proj_parts(hp + 1, qt)

    nc.compile()
    _BUILT = (nc,)
    return _BUILT


def make_in_maps(x, Wq, bq, Wk, bk):
    f16 = np.float16
    x = np.asarray(x, dtype=np.float32)
    wqT = np.ascontiguousarray(np.asarray(Wq, np.float32).T).astype(f16)
    wkT = np.ascontiguousarray(np.asarray(Wk, np.float32).T).astype(f16)
    bq = np.asarray(bq, np.float32)
    bk = np.asarray(bk, np.float32)
    in_maps = []
    for c in range(NCORES):
        b, qc = c // 4, c % 4
        xTb = np.ascontiguousarray(x[b].T).astype(f16)      # [768, 2048]
        xqTc = np.ascontiguousarray(xTb[:, qc * QPC:(qc + 1) * QPC])
        in_maps.append({
            "xT": xTb,
            "xqT": xqTc,
            "wqT": wqT,
            "wkT": wkT,
            "bq": bq,
            "bk": bk,
        })
    return in_maps


def run(x, Wq, bq, Wk, bk, trace=False, **trace_kwargs):
    from concourse.bass_utils import run_bass_kernel_spmd
    (nc,) = _build()
    in_maps = make_in_maps(x, Wq, bq, Wk, bk)
    res = run_bass_kernel_spmd(
        nc, in_maps, core_ids=list(range(NCORES)), trace=trace,
        **trace_kwargs)
    outp = np.zeros((B, S, S), np.float32)
    for c in range(NCORES):
        b, qc = c // 4, c % 4
        outp[b, qc * QPC:(qc + 1) * QPC, :] = \
            res.results[c]["out"].astype(np.float32)
    return outp, res


def kernel(x, Wq, bq, Wk, bk):
    outp, _ = run(x, Wq, bq, Wk, bk, trace=False)
    return outp
